# revision 21
# baseline (speedup 1.0000x reference)
"""GQA cross-attention block on 8 trn2 NeuronCores.

Sharding: tensor-parallel over heads. Core c owns KV group g=c (64 dims of
K/V) and its 4 query heads (256 q channels). Each core computes its heads'
attention plus its slice of the o-projection (rows c*256:(c+1)*256 of Wo),
producing a full-shape partial output; the host sums the 8 partials and
adds bo. No device collectives needed.

Key trick: 64-contraction matmuls run at 2 cycles/row on trn2, but two of
them with tile_position (0,0) and (64,0) execute CONCURRENTLY on the two
row-halves of the PE array (~2.8x measured). So kT is duplicated into both
partition halves and query heads are packed in pairs (head r_lo in
partitions 0:64, r_hi in 64:128); the two heads' score matmuls pair up.
One [128,1024] exp (ACT, two-bank PSUM read) covers both heads per
k-chunk. AV matmuls contract over the full 128 k positions. The softmax
epilogue (1/Z via SBUF-sourced reciprocal_approx_fast, fp16 ones-matmul
broadcast, DVE multiply) is deferred under the next unit's matmuls.
"""

import numpy as np
import ml_dtypes

import concourse.bass as bass
from concourse import bacc
import concourse.mybir as mybir
import concourse.tile as tile
from concourse.bass_utils import run_bass_kernel_spmd
from concourse.masks import make_identity

BF16 = ml_dtypes.bfloat16
F32 = mybir.dt.float32
F16 = mybir.dt.float16
BF = mybir.dt.bfloat16

B = 2
S = 2048
HID = 2048
D = 64          # head dim
RQ = 4          # query heads per core (per kv group)
CH = RQ * D     # 256 q channels per core
NCORES = 8
NH = HID // 128  # 16 hidden chunks
NST = S // 512   # 4 s-tiles of 512
NKC = S // 128   # 16 key chunks of 128
SCALE = 1.0 / np.sqrt(D)


def _build_nc() -> bass.Bass:
    nc = bacc.Bacc()

    xT = nc.dram_tensor("xT", [B, HID, S], BF, kind="ExternalInput")
    encT = nc.dram_tensor("encT", [B, HID, S], BF, kind="ExternalInput")
    wq = nc.dram_tensor("wq", [HID, CH], BF, kind="ExternalInput")
    wkv = nc.dram_tensor("wkv", [HID, 2 * D], BF, kind="ExternalInput")
    wo = nc.dram_tensor("wo", [CH, HID], BF, kind="ExternalInput")
    bq = nc.dram_tensor("bq", [CH, 1], F32, kind="ExternalInput")
    bkv = nc.dram_tensor("bkv", [2 * D, 1], F32, kind="ExternalInput")
    out = nc.dram_tensor("out", [B, S, HID], BF, kind="ExternalOutput")

    with tile.TileContext(nc) as tc:
        with (
            tc.tile_pool(name="wpool", bufs=1) as wpool,
            tc.tile_pool(name="xs", bufs=16) as xs_pool,
            tc.tile_pool(name="es", bufs=16) as es_pool,
            tc.tile_pool(name="acts", bufs=2) as acts,
            tc.tile_pool(name="vaug", bufs=2 * NKC) as vaug_pool,
            tc.tile_pool(name="epool", bufs=6) as epool,
            tc.tile_pool(name="small", bufs=4) as small,
            tc.tile_pool(name="osb", bufs=4) as osb_pool,
            tc.tile_pool(name="psum", bufs=2, space="PSUM") as ps,
        ):
            # ---- resident weights ----
            wq_t = []
            wkv_t = []
            for h in range(NH):
                wqh = wpool.tile([128, CH], BF, name=f"wq{h}")
                nc.scalar.dma_start(out=wqh[:], in_=wq[h * 128:(h + 1) * 128, :])
                wq_t.append(wqh)
                wkvh = wpool.tile([128, 2 * D], BF, name=f"wkv{h}")
                nc.scalar.dma_start(out=wkvh[:], in_=wkv[h * 128:(h + 1) * 128, :])
                wkv_t.append(wkvh)
            wo_t = []
            for cchunk in range(2):
                woc = wpool.tile([128, HID], BF, name=f"wo{cchunk}")
                nc.scalar.dma_start(out=woc[:], in_=wo[cchunk * 128:(cchunk + 1) * 128, :])
                wo_t.append(woc)
            bq_t = []
            for cchunk in range(2):
                bqc = wpool.tile([128, 1], F32, name=f"bq{cchunk}")
                nc.sync.dma_start(out=bqc[:], in_=bq[cchunk * 128:(cchunk + 1) * 128, :])
                bq_t.append(bqc)
            bkv_t = wpool.tile([2 * D, 1], F32, name="bkv_t")
            nc.sync.dma_start(out=bkv_t[:], in_=bkv[:, :])

            ident = wpool.tile([128, 128], BF, name="ident")
            make_identity(nc, ident[:])
            ones1 = wpool.tile([1, D], F16, name="ones1")
            nc.gpsimd.memset(ones1[:], 1.0)

            ID = mybir.ActivationFunctionType.Identity
            EXP = mybir.ActivationFunctionType.Exp

            # persistent v_aug tiles; ones column written once
            va_tiles = [
                [vaug_pool.tile([128, D + 1], BF, tag="vaug", name=f"va{b}_{kc}")
                 for kc in range(NKC)]
                for b in range(B)
            ]
            for b in range(B):
                for kc in range(NKC):
                    nc.gpsimd.memset(va_tiles[b][kc][:, D:D + 1], 1.0)

            # deferred softmax epilogues: (av_tile, row, qc, odst, b)
            pending = []
            pending_bc = []

            def flush_recip():
                # 1/Z chain on DVE; approx_fast needs an SBUF source
                while pending:
                    av, row, qc, odst, b = pending.pop(0)
                    key = f"{b}{row}{qc}{len(pending)}"
                    zs = small.tile([1, 512], F32, tag="zs", name=f"zs{key}")
                    nc.vector.tensor_copy(zs[:], av[D:D + 1, :])
                    rt = small.tile([1, 512], F32, tag="rt", name=f"rt{key}")
                    nc.vector.reciprocal_approx_fast(rt[:], zs[:])
                    rth = small.tile([1, 512], F16, tag="rth", name=f"rth{key}")
                    nc.vector.tensor_copy(rth[:], rt[:])
                    pending_bc.append((av, rth, row, qc, odst, b))

            def flush_bc():
                while pending_bc:
                    av, rth, row, qc, odst, b = pending_bc.pop(0)
                    qsl = slice(qc * 512, (qc + 1) * 512)
                    key = f"{b}{row}{qc}{len(pending_bc)}"
                    bcp = ps.tile([D, 512], F32, tag="big", name=f"bcp{key}")
                    nc.tensor.matmul(bcp[:], ones1[:], rth[:],
                                     start=True, stop=True)
                    bcs = small.tile([D, 512], F32, tag="bcs", name=f"bcs{key}")
                    nc.vector.tensor_copy(bcs[:], bcp[:])
                    nc.vector.tensor_mul(odst[row:row + D, qsl], av[0:D, :], bcs[:])

            def flush_epilogue():
                flush_recip()
                flush_bc()

            for b in range(B):
                # ---- projections ----
                # qp[0] holds heads 0,1 stacked on partitions (0:64, 64:128);
                # qp[1] holds heads 2,3. kT2 has kT duplicated in both halves.
                qp = [
                    acts.tile([128, S], BF, tag=f"qp{i}", name=f"qp{i}_{b}")
                    for i in range(2)
                ]
                kT2 = acts.tile([128, S], BF, tag="kT2", name=f"kT2{b}")
                vT = acts.tile([D, S], BF, tag="vT", name=f"vT{b}")

                def emit_vaug(st):
                    for j in range(4):
                        kc = st * 4 + j
                        vtp = ps.tile([128, D], BF, tag="pav", bufs=4,
                                      name=f"vtp{b}{kc}")
                        nc.tensor.transpose(
                            vtp[:], vT[:, kc * 128:(kc + 1) * 128], ident[0:D, 0:D])
                        nc.vector.tensor_copy(va_tiles[b][kc][:, 0:D], vtp[:])

                for st in range(NST):
                    ssl = slice(st * 512, (st + 1) * 512)
                    qps2 = ps.tile([128, 1024], F32, tag="big", name=f"qps{b}{st}")
                    kvps = ps.tile([128, 1024], F32, tag="big", name=f"kvps{b}{st}")
                    for h in range(NH):
                        xt = xs_pool.tile([128, 512], BF, tag="xs", name=f"xs{b}{st}{h}")
                        xeng = (nc.gpsimd, nc.scalar, nc.sync)[h % 3]
                        xeng.dma_start(
                            out=xt[:], in_=xT[b, h * 128:(h + 1) * 128, ssl])
                        et = es_pool.tile([128, 512], BF, tag="es", name=f"es{b}{st}{h}")
                        eeng = (nc.sync, nc.gpsimd, nc.scalar)[h % 3]
                        eeng.dma_start(
                            out=et[:], in_=encT[b, h * 128:(h + 1) * 128, ssl])
                        nc.tensor.matmul(
                            qps2[:, 0:512], wq_t[h][:, 0:128], xt[:],
                            start=(h == 0), stop=(h == NH - 1))
                        nc.tensor.matmul(
                            qps2[:, 512:1024], wq_t[h][:, 128:256], xt[:],
                            start=(h == 0), stop=(h == NH - 1))
                        nc.tensor.matmul(
                            kvps[:, 0:512], wkv_t[h][:], et[:],
                            start=(h == 0), stop=(h == NH - 1))
                        if h == 7 and st > 0:
                            emit_vaug(st - 1)
                    # full-height q copies first: they release the qps2
                    # slot that the next s-tile's matmuls wait on
                    nc.scalar.activation(
                        qp[0][:, ssl], qps2[:, 0:512], ID, bias=bq_t[0][:])
                    nc.scalar.activation(
                        qp[1][:, ssl], qps2[:, 512:1024], ID, bias=bq_t[1][:])
                    # kT duplicated into both halves; vT to partitions 0:64
                    nc.scalar.activation(
                        kT2[0:D, ssl], kvps[0:D, 0:512], ID, bias=bkv_t[0:D, :])
                    nc.scalar.activation(
                        kT2[D:128, ssl], kvps[0:D, 0:512], ID, bias=bkv_t[0:D, :])
                    nc.scalar.activation(
                        vT[:, ssl], kvps[D:2 * D, 0:512], ID, bias=bkv_t[D:2 * D, :])
                emit_vaug(NST - 1)

                # ---- attention: units of (qc, head pair), o-projection for
                # completed qc blocks interleaved into the exp-gated PE slack
                oT_lo = acts.tile([128, S], BF, tag="olo", name=f"olo{b}")
                oT_hi = acts.tile([128, S], BF, tag="ohi", name=f"ohi{b}")

                pending_oproj = []

                def emit_oproj(sc16, hp):
                    s128 = slice(sc16 * 128, (sc16 + 1) * 128)
                    ops = ps.tile([128, 1024], F32, tag="big",
                                  name=f"op{b}{sc16}{hp}")
                    for j in range(2):
                        hsl = slice(hp * 1024 + j * 512,
                                    hp * 1024 + (j + 1) * 512)
                        nc.tensor.matmul(
                            ops[:, j * 512:(j + 1) * 512],
                            oT_lo[:, s128], wo_t[0][:, hsl],
                            start=True, stop=False)
                        nc.tensor.matmul(
                            ops[:, j * 512:(j + 1) * 512],
                            oT_hi[:, s128], wo_t[1][:, hsl],
                            start=False, stop=True)
                    osb = osb_pool.tile([128, 1024], BF, tag="osb",
                                        name=f"ob{b}{sc16}{hp}")
                    # single-bank DVE reads only; ACT stays free for exp
                    nc.vector.tensor_copy(osb[:, 0:512], ops[:, 0:512])
                    nc.vector.tensor_copy(osb[:, 512:1024], ops[:, 512:1024])
                    nc.sync.dma_start(
                        out=out[b, s128, hp * 1024:(hp + 1) * 1024], in_=osb[:])

                for u in range(8):
                    qc, rpair = u // 2, u % 2
                    odst = oT_lo if rpair == 0 else oT_hi
                    qpt = qp[rpair]
                    qsl = slice(qc * 512, (qc + 1) * 512)
                    ava = ps.tile([D + 1, 512], F32, tag="pav", bufs=4,
                                  name=f"av{b}{rpair}{qc}a")
                    avb = ps.tile([D + 1, 512], F32, tag="pav", bufs=4,
                                  name=f"av{b}{rpair}{qc}b")
                    es = [None] * NKC
                    for kc in range(NKC):
                        ksl = slice(kc * 128, (kc + 1) * 128)
                        sct2 = ps.tile([128, 1024], F32, tag="big",
                                       name=f"sc{b}{rpair}{qc}{kc}")
                        # the two heads' score matmuls pair on the PE
                        # row-halves and run concurrently
                        nc.tensor.matmul(
                            sct2[:, 0:512], kT2[0:D, ksl], qpt[0:D, qsl],
                            start=True, stop=True)
                        nc.tensor.matmul(
                            sct2[:, 512:1024], kT2[D:128, ksl], qpt[D:128, qsl],
                            start=True, stop=True)
                        e2 = epool.tile([128, 1024], BF, tag="e",
                                        name=f"e{b}{rpair}{qc}{kc}")
                        nc.scalar.activation(
                            e2[:], sct2[:], EXP, scale=float(SCALE))
                        es[kc] = e2
                        # AV lags one k-chunk so exp latency never
                        # stalls the PE stream.
                        if kc > 0:
                            pe2 = es[kc - 1]
                            nc.tensor.matmul(
                                ava[:], va_tiles[b][kc - 1][:], pe2[:, 0:512],
                                start=(kc == 1), stop=False)
                            nc.tensor.matmul(
                                avb[:], va_tiles[b][kc - 1][:], pe2[:, 512:1024],
                                start=(kc == 1), stop=False)
                        if kc == 2:
                            flush_recip()
                        elif kc == 8:
                            flush_bc()
                            # the flush completed unit u-1's output rows; a
                            # finished qc block unlocks its o-projection
                            if u >= 1 and (u - 1) % 2 == 1:
                                qdone = (u - 1) // 2
                                for sc16 in range(qdone * 4, qdone * 4 + 4):
                                    pending_oproj.append((sc16, 0))
                                    pending_oproj.append((sc16, 1))
                        elif kc >= 9 and pending_oproj:
                            emit_oproj(*pending_oproj.pop(0))
                    pe2 = es[NKC - 1]
                    nc.tensor.matmul(
                        ava[:], va_tiles[b][NKC - 1][:], pe2[:, 0:512],
                        start=False, stop=True)
                    nc.tensor.matmul(
                        avb[:], va_tiles[b][NKC - 1][:], pe2[:, 512:1024],
                        start=False, stop=True)
                    pending.append((ava, 0, qc, odst, b))
                    pending.append((avb, D, qc, odst, b))

                flush_epilogue()
                for sc16 in range(12, 16):
                    pending_oproj.append((sc16, 0))
                    pending_oproj.append((sc16, 1))
                while pending_oproj:
                    emit_oproj(*pending_oproj.pop(0))

    if not nc.is_finalized():
        nc.finalize()
    return nc


_NC = None
_RUN_KWARGS = {}
_LAST_RESULT = None


def _get_nc():
    global _NC
    if _NC is None:
        _NC = _build_nc()
    return _NC


def kernel(x, encoder_output, Wq, bq, Wk, bk, Wv, bv, Wo, bo):
    nc = _get_nc()
    xT = np.ascontiguousarray(np.asarray(x, np.float32).transpose(0, 2, 1)).astype(BF16)
    encT = np.ascontiguousarray(
        np.asarray(encoder_output, np.float32).transpose(0, 2, 1)).astype(BF16)
    Wq = np.asarray(Wq, np.float32)
    Wk = np.asarray(Wk, np.float32)
    Wv = np.asarray(Wv, np.float32)
    Wo = np.asarray(Wo, np.float32)
    bk = np.asarray(bk, np.float32)
    bv = np.asarray(bv, np.float32)
    in_maps = []
    for c in range(NCORES):
        csl = slice(c * CH, (c + 1) * CH)
        gsl = slice(c * D, (c + 1) * D)
        in_maps.append({
            "xT": xT,
            "encT": encT,
            "wq": np.ascontiguousarray(Wq[:, csl]).astype(BF16),
            "wkv": np.ascontiguousarray(
                np.concatenate([Wk[:, gsl], Wv[:, gsl]], axis=1)).astype(BF16),
            "wo": np.ascontiguousarray(Wo[csl, :]).astype(BF16),
            "bq": np.ascontiguousarray(
                np.asarray(bq, np.float32)[csl].reshape(CH, 1)),
            "bkv": np.ascontiguousarray(
                np.concatenate([bk[gsl], bv[gsl]]).reshape(2 * D, 1)),
        })
    res = run_bass_kernel_spmd(nc, in_maps, list(range(NCORES)), **_RUN_KWARGS)
    global _LAST_RESULT
    _LAST_RESULT = res
    total = np.zeros((B, S, HID), np.float32)
    for c in range(NCORES):
        total += res.results[c]["out"].astype(np.float32)
    return total + np.asarray(bo, np.float32)
